# revision 10
# baseline (speedup 1.0000x reference)
"""Bass/Tile TRN2 kernel for nn_CausalSelfAttention_1116691497591.

Linear (softmax-free) bilinear attention returning (y, att):
    att = (q*m) @ (k*m)^T * SCALE            [B,H,T,T]   (the 1GB output)
    y   = (att @ (v*m)) reshaped @ Wp^T + bp [B,T,C]

Sharding: 8 cores = 4 batches x 2 query-halves (T split in two). Each core
computes att[b, :, half*1024:(half+1)*1024, :] and the matching y rows.
No cross-core reduction is needed: y's projection is computed per query row
via the associativity trick  y = (q*m*SCALE) @ (k_m^T v_m) @ Wp^T  which
turns the [T,T] @ [T,hd] contraction into two tiny [64,64]-rank updates.

All FLOPs run on device. The host only slices inputs, pre-transposes
k/Wp layouts (pure memory movement), and concatenates core outputs.
"""

import sys
from contextlib import ExitStack

import numpy as np

for _p in ("/opt/trn_rl_repo",):
    if _p not in sys.path:
        sys.path.append(_p)

import concourse.bass as bass  # noqa: E402
import concourse.tile as tile  # noqa: E402
from concourse import bacc, mybir  # noqa: E402
from concourse.bass_utils import run_bass_kernel_spmd  # noqa: E402
from concourse.masks import make_identity  # noqa: E402

B, T, C, H, HD = 4, 2048, 1024, 16, 64
SCALE = float(1.0 / np.sqrt(np.float32(HD)))
N_CORES = 8
TQ = T // 2  # query rows per core

F32 = mybir.dt.float32
F32R = mybir.dt.float32r
MM_DT = F32R  # fast fp32 matmul mode; set to F32 for full-precision fallback
MULT = mybir.AluOpType.mult
ADD = mybir.AluOpType.add


def _mm(x, dt=MM_DT):
    return x.bitcast(dt) if dt != F32 else x


def build_kernel(tc, aps, phases=("att", "mt", "g", "proj")):
    nc = tc.nc
    q, k, kT, x, wpt, mask_q, mask_kv, bp, att_o, y_o = aps
    ctx = ExitStack()
    with ctx:
        const = ctx.enter_context(tc.tile_pool(name="const", bufs=1))

        # --- constants / small tensors ---
        ident = const.tile([128, 128], F32, tag="ident")
        make_identity(nc, ident[:])

        mqs = const.tile([128, TQ // 128], F32, tag="mqs")  # mask_q * SCALE, tq on partitions
        nc.sync.dma_start(mqs[:], mask_q.rearrange("(o p) -> p o", p=128))
        nc.vector.tensor_scalar_mul(mqs[:], mqs[:], SCALE)

        mkv = const.tile([128, T // 128], F32, tag="mkv")  # mask_kv, t on partitions
        nc.sync.dma_start(mkv[:], mask_kv.rearrange("(o p) -> p o", p=128))

        mrow = const.tile([1, T], F32, tag="mrow")
        nc.sync.dma_start(mrow[:], mask_kv[None, :])
        mrep = const.tile([128, T], F32, tag="mrep")  # mask_kv replicated on all partitions
        nc.gpsimd.partition_broadcast(mrep[:], mrow[:])

        brow = const.tile([1, C], F32, tag="brow")
        nc.sync.dma_start(brow[:], bp[None, :])
        brep = const.tile([128, C], F32, tag="brep")  # bias replicated on all partitions
        nc.gpsimd.partition_broadcast(brep[:], brow[:])

        # qT[p, o, t] = q[t, o*128+p] * mask_q[t] * SCALE   (head h: partitions
        # 64*(h%2)+[0,64) of block o=h//2)
        qT = const.tile([128, C // 128, TQ], F32R, tag="qT")
        mt_sb = const.tile([128, 8, 256], F32, tag="mt_sb")  # M^T per head-pair block
        g_sb = const.tile([128, 8, C], F32R, tag="g_sb")  # G_h = M_h @ WpT_h rows

        # --- phase 1: load q, mask*scale, transpose on PE ---
        with tc.tile_pool(name="qin", bufs=2) as q_pool, tc.tile_pool(
            name="tp_ps", bufs=2, space="PSUM"
        ) as tp_ps:
            for o in range(TQ // 128):
                qin = q_pool.tile([128, C], F32, tag="qin")
                nc.sync.dma_start(qin[:], q[o * 128 : (o + 1) * 128, :])
                nc.vector.tensor_scalar_mul(qin[:], qin[:], mqs[:, o : o + 1])
                for cc in range(C // 128):
                    pt = tp_ps.tile([128, 128], F32, tag="tp")
                    nc.tensor.transpose(pt[:], qin[:, cc * 128 : (cc + 1) * 128], ident[:])
                    nc.vector.tensor_copy(qT[:, cc, o * 128 : (o + 1) * 128], pt[:])

        att_ps = ctx.enter_context(tc.tile_pool(name="att_ps", bufs=4, space="PSUM"))
        kt_pool = ctx.enter_context(tc.tile_pool(name="kt", bufs=2))
        asb_pool = ctx.enter_context(tc.tile_pool(name="asb", bufs=3))

        def att_block(g2):
            """att for head pair (2*g2, 2*g2+1): 16 matmuls per head."""
            kt2_in = kt_pool.tile([128, T], F32, tag="kt2_in")
            nc.sync.dma_start(kt2_in[:], kT[g2 * 128 : (g2 + 1) * 128, :])
            kt2 = kt_pool.tile([128, T], F32R, tag="kt2")
            nc.vector.tensor_tensor(kt2[:], kt2_in[:], mrep[:], MULT)
            for i2 in range(2):
                h = 2 * g2 + i2
                lo = 64 * i2
                for tqi in range(TQ // 128):
                    asb = asb_pool.tile([128, T], F32, tag="asb")
                    for tk in range(T // 512):
                        aps_t = att_ps.tile([128, 512], F32, tag="aps")
                        nc.tensor.matmul(
                            aps_t[:],
                            qT[lo : lo + 64, g2, tqi * 128 : (tqi + 1) * 128],
                            kt2[lo : lo + 64, tk * 512 : (tk + 1) * 512],
                            start=True,
                            stop=True,
                        )
                        if tk % 2 == 0:
                            nc.vector.tensor_copy(asb[:, tk * 512 : (tk + 1) * 512], aps_t[:])
                        else:
                            nc.scalar.copy(asb[:, tk * 512 : (tk + 1) * 512], aps_t[:])
                    nc.sync.dma_start(att_o[h, tqi * 128 : (tqi + 1) * 128, :], asb[:])

        if "att" not in phases:
            def att_block(g2):  # noqa: F811
                return

        # --- att first half, interleaved with M^T accumulation ---
        with tc.tile_pool(name="kx", bufs=2) as kx_pool, tc.tile_pool(
            name="mt_ps", bufs=1, space="PSUM"
        ) as mt_ps:
            # PSUM tiles are padded to a full 2KB bank; pack two head-pair
            # blocks ([128, 256] each) per bank -> 4 banks total.
            mt_banks = [
                mt_ps.tile([128, 512], F32, tag=f"mtb{i}", name=f"mtb{i}") for i in range(4)
            ]
            mt_tiles = [mt_banks[g // 2][:, (g % 2) * 256 : (g % 2) * 256 + 256] for g in range(8)]
            for g2 in range(4):
                att_block(g2)
                for ti in (range(4 * g2, 4 * g2 + 4) if "mt" in phases else ()):
                    kti = kx_pool.tile([128, C], F32, tag="kti")
                    nc.sync.dma_start(kti[:], k[ti * 128 : (ti + 1) * 128, :])
                    nc.vector.tensor_scalar_mul(kti[:], kti[:], mkv[:, ti : ti + 1])
                    xti = kx_pool.tile([128, C], F32, tag="xti")
                    nc.sync.dma_start(xti[:], x[ti * 128 : (ti + 1) * 128, :])
                    nc.vector.tensor_scalar_mul(xti[:], xti[:], mkv[:, ti : ti + 1])
                    for g in range(8):
                        # mt[g][d2, d1] = sum_t v[t, g*128+d2] * k[t, (g//2)*256+d1]
                        nc.tensor.matmul(
                            mt_tiles[g][:],
                            xti[:, g * 128 : (g + 1) * 128],
                            kti[:, (g // 2) * 256 : (g // 2 + 1) * 256],
                            start=(ti == 0 and g % 2 == 0),
                            stop=(ti == T // 128 - 1 and g % 2 == 1),
                        )
            if "mt" in phases:
                for g in range(8):
                    nc.vector.tensor_copy(mt_sb[:, g, :], mt_tiles[g][:])

        # --- G_h = M_h @ WpT[64h:64h+64, :]  (via lhsT = M^T block) ---
        with tc.tile_pool(name="wpt", bufs=2) as wpt_pool, tc.tile_pool(
            name="g_ps", bufs=2, space="PSUM"
        ) as g_ps:
            for g2 in range(8) if "g" in phases else ():
                wt = wpt_pool.tile([128, C], F32, tag="wt")
                nc.sync.dma_start(wt[:], wpt[g2 * 128 : (g2 + 1) * 128, :])
                gp = g_ps.tile([128, C], F32, tag="gp")
                for i2 in range(2):
                    h = 2 * g2 + i2
                    lo = 64 * i2
                    f0 = 64 * (h % 4)
                    for jh in range(2):
                        nc.tensor.matmul(
                            gp[lo : lo + 64, jh * 512 : (jh + 1) * 512],
                            mt_sb[lo : lo + 64, g2, f0 : f0 + 64],
                            wt[lo : lo + 64, jh * 512 : (jh + 1) * 512],
                            start=True,
                            stop=True,
                        )
                nc.vector.tensor_copy(g_sb[:, g2, :], gp[:])

        # --- att second half, interleaved with the projection ---
        with tc.tile_pool(name="yp", bufs=2) as y_pool, tc.tile_pool(
            name="pj_ps", bufs=2, space="PSUM"
        ) as pj_ps:

            def proj_block(tqi):
                # Accumulation chains must keep a constant PE row group: even
                # heads (array rows 0-63) accumulate in ppA, odd heads (rows
                # 64-127) in ppB; DVE combines A+B+bias.
                ysb = y_pool.tile([128, C], F32, tag="ysb")
                for jh in range(2):
                    ppA = pj_ps.tile([128, 512], F32, tag="ppA")
                    ppB = pj_ps.tile([128, 512], F32, tag="ppB")
                    for h in range(H):
                        lo = 64 * (h % 2)
                        pp = ppA if h % 2 == 0 else ppB
                        nc.tensor.matmul(
                            pp[:],
                            qT[lo : lo + 64, h // 2, tqi * 128 : (tqi + 1) * 128],
                            g_sb[lo : lo + 64, h // 2, jh * 512 : (jh + 1) * 512],
                            start=(h < 2),
                            stop=(h >= H - 2),
                        )
                    ysl = ysb[:, jh * 512 : (jh + 1) * 512]
                    # only one PSUM operand allowed per DVE op
                    nc.vector.tensor_tensor(ysl, ppA[:], brep[:, jh * 512 : (jh + 1) * 512], ADD)
                    nc.vector.tensor_tensor(ysl, ppB[:], ysl, ADD)
                nc.sync.dma_start(y_o[tqi * 128 : (tqi + 1) * 128, :], ysb[:])

            for g2 in range(4, 8):
                att_block(g2)
                for tqi in (
                    range(2 * (g2 - 4), 2 * (g2 - 4) + 2) if "proj" in phases else ()
                ):
                    proj_block(tqi)


_NC_CACHE = None


def _get_nc(phases=("att", "mt", "g", "proj")):
    global _NC_CACHE
    if _NC_CACHE is not None:
        return _NC_CACHE
    nc = bacc.Bacc("TRN2", target_bir_lowering=False, debug=False, num_devices=N_CORES)
    aps = (
        nc.dram_tensor("q", [TQ, C], F32, kind="ExternalInput").ap(),
        nc.dram_tensor("k", [T, C], F32, kind="ExternalInput").ap(),
        nc.dram_tensor("kT", [C, T], F32, kind="ExternalInput").ap(),
        nc.dram_tensor("x", [T, C], F32, kind="ExternalInput").ap(),
        nc.dram_tensor("wpt", [C, C], F32, kind="ExternalInput").ap(),
        nc.dram_tensor("mask_q", [TQ], F32, kind="ExternalInput").ap(),
        nc.dram_tensor("mask_kv", [T], F32, kind="ExternalInput").ap(),
        nc.dram_tensor("bp", [C], F32, kind="ExternalInput").ap(),
        nc.dram_tensor("att_o", [H, TQ, T], F32, kind="ExternalOutput").ap(),
        nc.dram_tensor("y_o", [TQ, C], F32, kind="ExternalOutput").ap(),
    )
    with tile.TileContext(nc) as tc:
        build_kernel(tc, aps, phases)
    nc.compile()
    _NC_CACHE = nc
    return nc


def make_in_maps(x, keys, queries, mask, Wp, bp):
    x = np.ascontiguousarray(np.asarray(x, dtype=np.float32))
    keys = np.ascontiguousarray(np.asarray(keys, dtype=np.float32))
    queries = np.ascontiguousarray(np.asarray(queries, dtype=np.float32))
    mask = np.ascontiguousarray(np.asarray(mask, dtype=np.float32))
    Wp = np.asarray(Wp, dtype=np.float32)
    bp = np.ascontiguousarray(np.asarray(bp, dtype=np.float32))
    kT = np.ascontiguousarray(keys.transpose(0, 2, 1))
    wpt = np.ascontiguousarray(Wp.T)
    in_maps = []
    for c in range(N_CORES):
        b, half = divmod(c, 2)
        sl = slice(half * TQ, (half + 1) * TQ)
        in_maps.append(
            {
                "q": np.ascontiguousarray(queries[b, sl]),
                "k": keys[b],
                "kT": kT[b],
                "x": x[b],
                "wpt": wpt,
                "mask_q": np.ascontiguousarray(mask[b, sl]),
                "mask_kv": mask[b],
                "bp": bp,
            }
        )
    return in_maps


def assemble(results):
    att = np.empty((B, H, T, T), np.float32)
    y = np.empty((B, T, C), np.float32)
    for c in range(N_CORES):
        b, half = divmod(c, 2)
        sl = slice(half * TQ, (half + 1) * TQ)
        att[b, :, sl, :] = results[c]["att_o"]
        y[b, sl, :] = results[c]["y_o"]
    return y, att


def kernel(x, keys, queries, mask, Wp, bp):
    nc = _get_nc()
    in_maps = make_in_maps(x, keys, queries, mask, Wp, bp)
    res = run_bass_kernel_spmd(nc, in_maps, core_ids=list(range(N_CORES)))
    return assemble(res.results)
